# revision 12
# baseline (speedup 1.0000x reference)
"""Trainium2 Bass kernel for the DNPU local-receptive-field surrogate model.

Model (see reference): x [B,1,64,64] -> 2x2/stride-2 unfold -> per-node
7-electrode assembly (4 data + 3 control electrodes, placements given by
data_idx/ctrl_idx) -> shared MLP 7->90->(90x4)->1 -> out [B,32,32].

Strategy:
  - Data-parallel over batch: 64 batches per core x 8 cores; tiny weights
    and per-node controls replicated to every core.
  - On-chip layout: hidden units on SBUF partitions, tokens (b,n) on the
    free dim; each matmul processes 512 tokens (one PSUM bank).
  - The whole unfold + electrode scatter + layer-0 contraction is ONE
    K=7 matmul per tile: the host pre-shuffles x into pixel-major layout
    (partition p = patch pixel p, contiguous tokens on the free dim) with
    the per-node control rows tiled into partitions 4-6 of the same
    buffer; the stationary operand stacks the gathered W_in rows for the
    4 data + 3 control electrodes.
  - LAYER-MAJOR sweeps: within a chunk of 16 batches (32 tiles), all
    matmuls of layer L run back-to-back before layer L+1. Measured on
    this hw: a 512-column bf16 matmul issues every 426.7ns = 1 column
    per 1.2GHz cycle, *independent of dtype and forever* (the cost
    model's 2.4GHz p-state never engages on hardware -- verified with
    2000 dependency-free back-to-back matmuls; fp8 DoubleRow is both
    ISA-rejected for this geometry and far outside the 2e-2 error
    budget, ~12% rel err). So the kernel is PE-column-rate-bound at
    6*65536 columns/core = 327.7us; everything else engineers the
    overhead around that floor:
      * 16-batch chunks (4 chunks/core) halve the number of sweep/chunk
        boundaries vs 8-batch chunks -- each boundary costs ~0.5-1.1us
        in extra Tensor-queue semaphore ops.
      * chunk x is one host-packed [7, chunk_tok] buffer (pixel rows 0-3
        + tiled control rows 4-6), one DMA per chunk, prefetched right
        after the *previous* chunk's layer-0 sweep (4 sweeps of runway;
        the transfer is ~12us serial over only 7 partition lines).
        Chunk 0 is staged as a small head piece (first 2 tiles) +
        remainder so the first matmul starts as soon as ~14KB land.
      * out-layer matmuls run as ONE block at the end of the l4 sweep
        (single wh4->wo stationary switch), in GROUPS OF 4 that write
        partitions {0,32,64,96} of one PSUM slot via the matmul
        tile_position column-block: one [97,512] drain retires 4 out
        tiles (engines are free-dim-serial, so this quarters both the
        out-drain count and, critically, spreads the chunk output over
        4 SBUF partition lines -- the old [1, chunk_tok] layout made
        the final output DMA a single-partition ~25us serial transfer
        that dominated the kernel tail and queue-blocked the next
        chunk's x).
  - ONE unified PSUM pool: an 8-deep ring of single-bank [97,512] slots
    covering all 8 banks (hidden matmul -> one slot -> one drain; out
    group: 4 matmuls -> one slot -> one drain). Each hidden drain fuses
    bias+ReLU (ACT activation or DVE tensor_scalar add+max, greedily
    balanced by modeled per-op cost); ACT/DVE are the only engines that
    can read PSUM. b_out is broadcast host-side to a [97,1] bias tensor.
"""

import ml_dtypes
import numpy as np

import concourse.bass as bass
import concourse.mybir as mybir
import concourse.tile as _tile
from concourse.bass_utils import run_bass_kernel_spmd

# ---------------------------------------------------------------------------
# Workaround: this neuronxcc walrus build rejects instructions carrying more
# than one sem wait ("Too many sync wait commands"; ACT and SP DMA reject
# even 2). Tile freely attaches several waits to one instruction (and its
# tail drain waits on every proc sem at once). After scheduling, spill
# excess waits onto NOPs inserted just before the instruction on the same
# engine — engines execute their stream in order, so semantics are
# unchanged.
_MAX_SYNC_WAITS = 1
_nop_counter = [0]


def _split_excess_sync_waits(nc, maxw=_MAX_SYNC_WAITS):
    for f in nc.m.functions:
        for bb in f.blocks:
            insts = list(bb.instructions)
            if not any(
                ins.sync_info is not None and len(ins.sync_info.on_wait or []) > maxw
                for ins in insts
            ):
                continue
            new = []
            for ins in insts:
                si = ins.sync_info
                waits = list(si.on_wait or []) if si is not None else []
                if len(waits) > maxw:
                    excess, keep = waits[: len(waits) - maxw], waits[-maxw:]
                    for i in range(0, len(excess), maxw):
                        _nop_counter[0] += 1
                        nop = mybir.InstNoOp(name=f"waitsplit_{_nop_counter[0]}")
                        nop.engine = ins.engine
                        nop.sync_info = mybir.SyncInfo(
                            on_wait=excess[i : i + maxw], on_update=[]
                        )
                        new.append(nop)
                    si.on_wait = keep
                new.append(ins)
            bb.instructions = new

# ---------------------------------------------------------------------------
# Problem constants (hardcoded per the task contract).
B = 512
H = W = 64
K = 2
N_NODES = (H // K) * (W // K)  # 1024
HID = 90
N_HIDDEN = 4
N_CORES = 8
B_CORE = B // N_CORES  # 64 batches per core

CHUNK_B = 16  # batches per layer-major chunk
N_TILE = 512  # tokens per matmul (one PSUM bank of fp32)
HEAD_TILES = 4  # first-chunk tiles staged in the small head DMA
OGRP = 4  # out tiles per PSUM slot (partition blocks 0/32/64/96)

F32 = mybir.dt.float32
BF16 = mybir.dt.bfloat16  # matmul operand dtype (PSUM accum stays fp32)

# modeled drain costs (ns) for the greedy ACT/DVE balancer
_COST = {
    "act_out": 602.0,
    "dve_out": 660.0,
    "act_pair": 1038.0,
    "dve_pair": 1192.0,
}


def _build_program(b_core: int, chunk_b: int):
    """Trace the per-core Bass program (identical on all 8 cores)."""
    nc = bass.Bass()

    n_chunks = b_core // chunk_b
    chunk_tok = chunk_b * N_NODES
    tiles = chunk_tok // N_TILE  # matmul tiles per layer sweep (32)
    pairs = tiles // 2
    ogroups = tiles // OGRP  # out groups per chunk (8)
    head_tok = HEAD_TILES * N_TILE

    # host-packed chunk input: [7, b_core*1024]; partitions 0-3 = patch
    # pixel p, partitions 4-6 = tiled per-node controls
    xs_d = nc.dram_tensor("xs", [7, b_core * N_NODES], BF16, kind="ExternalInput")
    wz_d = nc.dram_tensor("wz", [7, HID], BF16, kind="ExternalInput")
    wh_d = nc.dram_tensor("wh", [HID, N_HIDDEN, HID], BF16, kind="ExternalInput")
    wo_d = nc.dram_tensor("wo", [HID, 1], BF16, kind="ExternalInput")
    bia_d = nc.dram_tensor("bia", [HID, 5], F32, kind="ExternalInput")
    boc_d = nc.dram_tensor("boc", [97, 1], F32, kind="ExternalInput")
    out_d = nc.dram_tensor("out", [b_core, N_NODES], F32, kind="ExternalOutput")

    Relu = mybir.ActivationFunctionType.Relu
    Identity = mybir.ActivationFunctionType.Identity
    ALU_ADD = mybir.AluOpType.add
    ALU_MAX = mybir.AluOpType.max

    eng_t = [0.0, 0.0]  # modeled busy ns: [ACT, DVE]

    with _tile.TileContext(nc) as tc:
        with (
            tc.tile_pool(name="const", bufs=1) as const,
            tc.tile_pool(name="xin", bufs=1) as xin,
            tc.tile_pool(name="outp", bufs=ogroups) as outp,
            tc.tile_pool(name="hbuf", bufs=pairs + 4) as hbuf,
            tc.tile_pool(name="ps", bufs=4, space="PSUM") as ps,
        ):
            # ---- first DMAs: exactly what matmul 0 needs, smallest first.
            # Each dma_start costs ~0.8us of SP-queue issue time, so the
            # order here directly sets when the PE can start.
            wz = const.tile([7, HID], BF16)
            nc.sync.dma_start(wz[:], wz_d[:])
            # chunk-0 x arrives in 3 pieces so layer-0 can start early
            # (the DMA line rate only slightly outruns l0 streaming, and a
            # piece's semaphore fires on full completion).
            xt = xin.tile([7, chunk_tok], BF16, tag="xt")
            bounds = [0, 4, 8, 14, 22, 32]
            for a, b in zip(bounds, bounds[1:]):
                nc.sync.dma_start(
                    xt[:, a * N_TILE : b * N_TILE],
                    xs_d[:, a * N_TILE : b * N_TILE],
                )
            bia = const.tile([HID, 5], F32)
            nc.sync.dma_start(bia[:], bia_d[:])
            wh = const.tile([HID, N_HIDDEN, HID], BF16)
            nc.sync.dma_start(wh[:], wh_d[:])
            wo = const.tile([HID, 1], BF16)
            nc.sync.dma_start(wo[:], wo_d[:])
            boc = const.tile([97, 1], F32)
            nc.sync.dma_start(boc[:], boc_d[:])

            def drain(dst, src, bias_ap, relu, kind="pair", eng=None):
                """PSUM->SBUF bias+(relu). eng forces ACT(0)/DVE(1); default
                picks the modeled-least-busy engine."""
                ca, cd = _COST[f"act_{kind}"], _COST[f"dve_{kind}"]
                if eng is None:
                    use_act = eng_t[0] + ca <= eng_t[1] + cd
                else:
                    use_act = eng == 0
                if use_act:
                    eng_t[0] += ca
                    nc.scalar.activation(
                        dst, src, Relu if relu else Identity, bias=bias_ap
                    )
                elif relu:
                    eng_t[1] += cd
                    nc.vector.tensor_scalar(
                        out=dst,
                        in0=src,
                        scalar1=bias_ap,
                        scalar2=0.0,
                        op0=ALU_ADD,
                        op1=ALU_MAX,
                    )
                else:
                    eng_t[1] += cd
                    nc.vector.tensor_scalar(
                        out=dst,
                        in0=src,
                        scalar1=bias_ap,
                        scalar2=None,
                        op0=ALU_ADD,
                    )

            for ck in range(n_chunks):
                b0 = ck * chunk_b
                t0 = b0 * N_NODES

                # ---- layer 0 sweep: one K=7 matmul per tile
                hs = []
                for pr in range(pairs):
                    h = hbuf.tile([HID, 2 * N_TILE], BF16, tag="h")
                    pt = ps.tile([97, 2 * N_TILE], F32, tag="ps")
                    for half in range(2):
                        c0 = (2 * pr + half) * N_TILE
                        nc.tensor.matmul(
                            pt[0:HID, half * N_TILE : (half + 1) * N_TILE],
                            wz[:],
                            xt[:, c0 : c0 + N_TILE],
                        )
                    drain(h[:], pt[0:HID, :], bia[:, 0:1], relu=True)
                    hs.append(h)

                # prefetch next chunk's x now: the transfer is ~12us over 7
                # partition lines and the l1-l4 sweeps (~55us) hide it; the
                # WAR on this chunk's layer-0 reads has just resolved.
                if ck + 1 < n_chunks:
                    nt0 = (ck + 1) * chunk_b * N_NODES
                    nc.sync.dma_start(xt[:], xs_d[:, nt0 : nt0 + chunk_tok])

                # ---- hidden layer sweeps
                for li in range(N_HIDDEN):
                    hs2 = []
                    for pr in range(pairs):
                        h = hbuf.tile([HID, 2 * N_TILE], BF16, tag="h")
                        pt = ps.tile([97, 2 * N_TILE], F32, tag="ps")
                        for half in range(2):
                            nc.tensor.matmul(
                                pt[0:HID, half * N_TILE : (half + 1) * N_TILE],
                                wh[:, li, :],
                                hs[pr][:, half * N_TILE : (half + 1) * N_TILE],
                            )
                        drain(
                            h[:], pt[0:HID, :], bia[:, li + 1 : li + 2], relu=True
                        )
                        hs2.append(h)
                    hs = hs2

                # ---- out sweep: groups of 4 M=1 matmuls into partition
                # blocks {0,32,64,96} of one PSUM slot (inputs all drained
                # sweeps ago), one [97,512] drain + one 4-line DMA per group
                for gp in range(ogroups // 2):
                    pt = ps.tile([97, 2 * N_TILE], F32, tag="ps")
                    for sh in range(2):  # slot half = out group
                        for j in range(OGRP):
                            t = gp * 2 * OGRP + sh * OGRP + j
                            pr, half = t // 2, t % 2
                            nc.tensor.matmul(
                                pt[32 * j : 32 * j + 1,
                                   sh * N_TILE : (sh + 1) * N_TILE],
                                wo[:],
                                hs[pr][:, half * N_TILE : (half + 1) * N_TILE],
                                tile_position=(0, 32 * j),
                            )
                    og = outp.tile([97, 2 * N_TILE], F32, tag="o")
                    drain(og[:], pt[:], boc[:], relu=False)
                    # slot half sh, block j -> token tile gp*8 + sh*4 + j;
                    # dram dst viewed [sh, j, 512] must pair with src dims
                    # (j: partition stride 32, sh: free half, c)
                    dst = (
                        out_d[b0 + 4 * gp : b0 + 4 * gp + 4]
                        .rearrange(
                            "b (r c) -> (b r) c", r=2, c=N_TILE
                        )  # [8 tiles, 512]
                        .rearrange("(s j) c -> j s c", s=2, j=OGRP)
                    )
                    nc.sync.dma_start(
                        dst,
                        og[0:97:32, :].rearrange("j (s c) -> j s c", s=2, c=N_TILE),
                    )

    _split_excess_sync_waits(nc)
    return nc


def _prep_weights(controls, W_in, b_in, W_h, b_h, W_out, b_out, data_idx, ctrl_idx):
    """Host-side prep: gather W_in rows per electrode placement (replicating
    the reference's scatter semantics)."""
    di = np.asarray(data_idx)[0].tolist()  # placements identical across nodes
    ci = np.asarray(ctrl_idx)[0].tolist()
    W_in = np.asarray(W_in, dtype=np.float32)
    Wd = W_in[di, :].copy()  # [4, HID]
    cset = set(ci)
    for j in range(4):
        if di[j] in cset or di[j] in di[j + 1 :]:
            Wd[j] = 0.0  # overwritten by a control (or a later data) electrode
    Wc = W_in[ci, :].copy()  # [3, HID]
    for k in range(3):
        if ci[k] in ci[k + 1 :]:
            Wc[k] = 0.0  # later control write wins

    bf = ml_dtypes.bfloat16
    common = {
        "wz": np.ascontiguousarray(
            np.concatenate([Wd, Wc], axis=0).astype(bf)
        ),  # [7, HID]
        "wh": np.ascontiguousarray(
            np.asarray(W_h, np.float32).astype(bf).transpose(1, 0, 2)
        ),
        "wo": np.ascontiguousarray(np.asarray(W_out, np.float32).astype(bf)),
        "bia": np.ascontiguousarray(
            np.concatenate(
                [np.asarray(b_in, np.float32)[:, None],
                 np.asarray(b_h, np.float32).T],
                axis=1,
            )
        ),
        "boc": np.full((97, 1), np.asarray(b_out, np.float32).ravel()[0],
                       dtype=np.float32),
    }
    return common


def _pack_x(x_core, ctl_rows):
    """[b,64,64] + [3, 1024] -> [7, b*1024]: partitions 0-3 = patch pixel
    (kh*2+kw), partitions 4-6 = controls tiled per batch; tokens (b, node)
    on the free dim."""
    b = x_core.shape[0]
    p = x_core.reshape(b, 32, 2, 32, 2).transpose(2, 4, 0, 1, 3)
    out = np.empty((7, b * N_NODES), dtype=ml_dtypes.bfloat16)
    out[0:4] = p.reshape(4, b * N_NODES).astype(ml_dtypes.bfloat16)
    out[4:7] = np.tile(ctl_rows, (1, b))
    return np.ascontiguousarray(out)


def _run(inputs, trace=False, tmpdir=None):
    x = np.asarray(inputs["x"], dtype=np.float32)
    common = _prep_weights(
        inputs["controls"],
        inputs["W_in"],
        inputs["b_in"],
        inputs["W_h"],
        inputs["b_h"],
        inputs["W_out"],
        inputs["b_out"],
        inputs["data_idx"],
        inputs["ctrl_idx"],
    )
    ctl_rows = (
        np.asarray(inputs["controls"], np.float32).T.astype(ml_dtypes.bfloat16)
    )  # [3, 1024]

    nc = _build_program(B_CORE, CHUNK_B)

    core_ids = list(range(N_CORES))
    in_maps = []
    for i in core_ids:
        shard = _pack_x(x[i * B_CORE : (i + 1) * B_CORE, 0], ctl_rows)
        in_maps.append({"xs": shard, **common})

    res = run_bass_kernel_spmd(nc, in_maps, core_ids, trace=trace, tmpdir=tmpdir)
    out = np.concatenate([res.results[i]["out"] for i in core_ids], axis=0)
    return out.reshape(B, 32, 32), res.exec_time_ns


def kernel(**inputs):
    return _run(inputs, trace=False)[0]


# revision 14
# speedup vs baseline: 1.1563x; 1.1563x over previous
"""Trainium2 Bass kernel for the DNPU local-receptive-field surrogate model.

Model (see reference): x [B,1,64,64] -> 2x2/stride-2 unfold -> per-node
7-electrode assembly -> shared MLP 7->90->(90x4)->1 -> out [B,32,32].

Measured hardware facts driving the design (this axon-tunneled TRN2):
  - A bf16 matmul streams 1 moving column per 1.2GHz cycle (426.7ns per
    512-column tile), regardless of dtype, M, K, and runtime (no 2.4GHz
    p-state ever engages; verified with 2000 dependency-free matmuls).
  - Matmuls whose outputs sit in DIFFERENT 32-partition PSUM column
    blocks (matmul tile_position=(0, 32j)) execute CONCURRENTLY; tiles
    sharing a column block serialize. So PE throughput = columns per
    block-saturated schedule, NOT per instruction.
  - ACT/DVE (the only PSUM readers) are free-dim-serial: a [128,512]
    drain costs the same as [1,512] (~620ns), so drains want ALL used
    partitions per instruction.
  - fp8 DoubleRow is ISA-rejected for this geometry and ~12% rel err
    anyway (budget 2e-2); bf16 keeps rel err at 6.4e-3.

Strategy:
  - Data-parallel over batch: 64 batches/core x 8 cores; weights and
    per-node controls replicated; host pre-packs x into pixel-major
    [7, tokens] (4 patch pixels + 3 tiled control rows) so unfold +
    electrode scatter + layer-0 is one K=7 matmul per 512-token tile.
  - SOFTWARE-PIPELINED BLOCK-ROTATED SWEEPS: each relu layer s places
    its 90 output rows on 3 of the 4 PSUM column blocks, rotated per
    layer (ROT[s]); the next layer's stationary is host-permuted to
    [128, 90] with zero rows where the producer layout has garbage, so
    K=128 matmuls consume the placed layout directly and a single
    [128, 512] drain (placed bias + relu; junk rows drain harmlessly,
    zero stationary rows kill them downstream) retires each tile.
    A per-chunk round-robin emitter interleaves all 5 layer sweeps
    (+ the M=1 out matmuls, 4 per block-packed PSUM slot) with a lag,
    so on average all 4 column blocks stay loaded: per-block demand is
    15 layer-parts + out spread over 4 blocks -> ~342ns/tile-layer vs
    427 serialized, on top of the out layer costing ~1/4 of a sweep.
  - Out groups: 4 M=1 matmuls -> partitions {0,32,64,96} of one PSUM
    slot -> one [97,512] drain -> one 4-partition-line DMA (the old
    [1, chunk_tok] single-line output DMA was a ~25us serial transfer
    that dominated the tail).
  - Chunk-0 x arrives in staged pieces so layer 0 starts once ~28KB
    land; later chunks prefetch after layer 0 stops reading xt.
"""

import ml_dtypes
import numpy as np

import concourse.bass as bass
import concourse.mybir as mybir
import concourse.tile as _tile
from concourse.bass_utils import run_bass_kernel_spmd

# ---------------------------------------------------------------------------
# Workaround: this neuronxcc walrus build rejects instructions carrying more
# than one sem wait ("Too many sync wait commands"). Spill excess waits onto
# NOPs inserted just before the instruction on the same engine.
_MAX_SYNC_WAITS = 1
_nop_counter = [0]


def _split_excess_sync_waits(nc, maxw=_MAX_SYNC_WAITS):
    for f in nc.m.functions:
        for bb in f.blocks:
            insts = list(bb.instructions)
            if not any(
                ins.sync_info is not None and len(ins.sync_info.on_wait or []) > maxw
                for ins in insts
            ):
                continue
            new = []
            for ins in insts:
                si = ins.sync_info
                waits = list(si.on_wait or []) if si is not None else []
                if len(waits) > maxw:
                    excess, keep = waits[: len(waits) - maxw], waits[-maxw:]
                    for i in range(0, len(excess), maxw):
                        _nop_counter[0] += 1
                        nop = mybir.InstNoOp(name=f"waitsplit_{_nop_counter[0]}")
                        nop.engine = ins.engine
                        nop.sync_info = mybir.SyncInfo(
                            on_wait=excess[i : i + maxw], on_update=[]
                        )
                        new.append(nop)
                    si.on_wait = keep
                new.append(ins)
            bb.instructions = new

# ---------------------------------------------------------------------------
B = 512
H = W = 64
K = 2
N_NODES = (H // K) * (W // K)  # 1024
HID = 90
N_HIDDEN = 4
N_STAGES = 1 + N_HIDDEN  # relu layers (l0..l4)
N_CORES = 8
B_CORE = B // N_CORES  # 64 batches per core

CHUNK_B = 16
N_TILE = 512
OGRP = 4  # out tiles per PSUM slot

# per-relu-layer column blocks for parts P0 (hid 0:32), P1 (32:64),
# P2 (64:90); rotated so the 15 parts spread over 4 blocks
ROT = [[0, 1, 2], [3, 0, 1], [2, 3, 0], [1, 2, 3], [0, 1, 2]]
PART_W = [32, 32, HID - 64]  # part widths
PART_H0 = [0, 32, 64]  # hid range starts
LAG_T = 5  # consumer stage trails producer by this many tiles

F32 = mybir.dt.float32
BF16 = mybir.dt.bfloat16

_COST = {
    "act_out": 602.0,
    "dve_out": 660.0,
}


def _hid_of_row(s):
    """For relu layer s's output layout: psum row r -> hid index or None."""
    m = [None] * 128
    for j, blk in enumerate(ROT[s]):
        for k in range(PART_W[j]):
            m[32 * blk + k] = PART_H0[j] + k
    return m


def _build_program(b_core: int, chunk_b: int):
    nc = bass.Bass()

    n_chunks = b_core // chunk_b
    chunk_tok = chunk_b * N_NODES
    tiles = chunk_tok // N_TILE  # 32
    ogroups = tiles // OGRP  # 8

    xs_d = nc.dram_tensor("xs", [7, b_core * N_NODES], BF16, kind="ExternalInput")
    wz_d = nc.dram_tensor("wz", [7, HID], BF16, kind="ExternalInput")
    # permuted hidden stationaries [128, 4, 90]
    wh_d = nc.dram_tensor("wh", [128, N_HIDDEN, HID], BF16, kind="ExternalInput")
    wo_d = nc.dram_tensor("wo", [HID, 1], BF16, kind="ExternalInput")
    # placed biases [128, 5] (col s = relu layer s, rows per ROT[s])
    bia_d = nc.dram_tensor("bia", [128, N_STAGES], F32, kind="ExternalInput")
    boc_d = nc.dram_tensor("boc", [97, 1], F32, kind="ExternalInput")
    out_d = nc.dram_tensor("out", [b_core, N_NODES], F32, kind="ExternalOutput")

    Relu = mybir.ActivationFunctionType.Relu
    Identity = mybir.ActivationFunctionType.Identity
    ALU_ADD = mybir.AluOpType.add
    ALU_MAX = mybir.AluOpType.max

    eng_t = [0.0, 0.0]

    with _tile.TileContext(nc) as tc:
        with (
            tc.tile_pool(name="const", bufs=1) as const,
            tc.tile_pool(name="xin", bufs=1) as xin,
            tc.tile_pool(name="outp", bufs=2 * ogroups) as outp,
            tc.tile_pool(name="hbuf", bufs=60) as hbuf,
            tc.tile_pool(name="ps", bufs=8, space="PSUM") as ps,
        ):
            wz = const.tile([7, HID], BF16)
            nc.sync.dma_start(wz[:], wz_d[:])
            xt = xin.tile([7, chunk_tok], BF16, tag="xt")
            bounds = [0, 4, 8, 14, 22, 32]
            for a, b in zip(bounds, bounds[1:]):
                nc.sync.dma_start(
                    xt[:, a * N_TILE : b * N_TILE],
                    xs_d[:, a * N_TILE : b * N_TILE],
                )
            bia = const.tile([128, N_STAGES], F32)
            nc.sync.dma_start(bia[:], bia_d[:])
            wh = const.tile([128, N_HIDDEN, HID], BF16)
            nc.sync.dma_start(wh[:], wh_d[:])
            wo = const.tile([HID, 1], BF16)
            nc.sync.dma_start(wo[:], wo_d[:])
            boc = const.tile([97, 1], F32)
            nc.sync.dma_start(boc[:], boc_d[:])

            def drain(dst, src, bias_ap, relu, eng=None):
                ca, cd = _COST["act_out"], _COST["dve_out"]
                if eng is None:
                    use_act = eng_t[0] + ca <= eng_t[1] + cd
                else:
                    use_act = eng == 0
                if use_act:
                    eng_t[0] += ca
                    nc.scalar.activation(
                        dst, src, Relu if relu else Identity, bias=bias_ap
                    )
                elif relu:
                    eng_t[1] += cd
                    nc.vector.tensor_scalar(
                        out=dst, in0=src, scalar1=bias_ap, scalar2=0.0,
                        op0=ALU_ADD, op1=ALU_MAX,
                    )
                else:
                    eng_t[1] += cd
                    nc.vector.tensor_scalar(
                        out=dst, in0=src, scalar1=bias_ap, scalar2=None,
                        op0=ALU_ADD,
                    )

            for ck in range(n_chunks):
                b0 = ck * chunk_b
                hcur = [[None] * tiles for _ in range(N_STAGES)]

                def emit_stage_tile(s, t, hcur=hcur, xt=xt):
                    """3 part-matmuls + 1 drain for relu layer s, tile t."""
                    pt = ps.tile([128, N_TILE], F32, tag="ps", name="pt_s")
                    for j, blk in enumerate(ROT[s]):
                        w = PART_W[j]
                        if s == 0:
                            lhs = wz[:, PART_H0[j] : PART_H0[j] + w]
                            rhs = xt[:, t * N_TILE : (t + 1) * N_TILE]
                        else:
                            lhs = wh[:, s - 1, PART_H0[j] : PART_H0[j] + w]
                            rhs = hcur[s - 1][t][:]
                        nc.tensor.matmul(
                            pt[32 * blk : 32 * blk + w, :],
                            lhs,
                            rhs,
                            tile_position=(0, 32 * blk),
                        )
                    h = hbuf.tile([128, N_TILE], BF16, tag="h", name="h_s")
                    drain(h[:], pt[:], bia[:, s : s + 1], relu=True)
                    hcur[s][t] = h

                def emit_out_group(g, hcur=hcur, b0=b0):
                    """4 M=1 out matmuls -> one psum slot -> drain + DMA."""
                    pt = ps.tile([128, N_TILE], F32, tag="ps", name="pt_o")
                    for j in range(OGRP):
                        t = g * OGRP + j
                        nc.tensor.matmul(
                            pt[32 * j : 32 * j + 1, :],
                            wo[:],
                            hcur[N_STAGES - 1][t][0:HID, :],
                            tile_position=(0, 32 * j),
                        )
                    og = outp.tile([97, N_TILE], F32, tag="o", name="og")
                    drain(og[:], pt[0:97, :], boc[:], relu=False)
                    dst = (
                        out_d[b0 + 2 * g : b0 + 2 * g + 2]
                        .rearrange("b (r c) -> (b r) c", r=2, c=N_TILE)
                    )
                    nc.sync.dma_start(dst, og[0:97:32, :])

                # ---- round-robin pipelined emission
                p = [0] * N_STAGES
                po = 0
                prefetched = ck + 1 >= n_chunks
                while p[-1] < tiles or po < ogroups:
                    progress = False
                    for s in range(N_STAGES):
                        limit = tiles if s == 0 else p[s - 1] - LAG_T
                        if p[s] < tiles and p[s] < limit:
                            emit_stage_tile(s, p[s])
                            p[s] += 1
                            progress = True
                    if not prefetched and p[0] >= tiles:
                        nt0 = (ck + 1) * chunk_b * N_NODES
                        nc.sync.dma_start(
                            xt[:], xs_d[:, nt0 : nt0 + chunk_tok]
                        )
                        prefetched = True
                    if po < ogroups and (po + 1) * OGRP <= p[-1] - LAG_T:
                        emit_out_group(po)
                        po += 1
                        progress = True
                    if not progress:
                        for s in range(N_STAGES):
                            if p[s] < tiles and (s == 0 or p[s] < p[s - 1]):
                                emit_stage_tile(s, p[s])
                                p[s] += 1
                                break
                        else:
                            if po < ogroups:
                                emit_out_group(po)
                                po += 1

    _split_excess_sync_waits(nc)
    return nc


def _prep_weights(controls, W_in, b_in, W_h, b_h, W_out, b_out, data_idx, ctrl_idx):
    di = np.asarray(data_idx)[0].tolist()
    ci = np.asarray(ctrl_idx)[0].tolist()
    W_in = np.asarray(W_in, dtype=np.float32)
    Wd = W_in[di, :].copy()
    cset = set(ci)
    for j in range(4):
        if di[j] in cset or di[j] in di[j + 1 :]:
            Wd[j] = 0.0
    Wc = W_in[ci, :].copy()
    for k in range(3):
        if ci[k] in ci[k + 1 :]:
            Wc[k] = 0.0

    bf = ml_dtypes.bfloat16
    W_h = np.asarray(W_h, np.float32)  # [4, 90, 90]

    # permuted hidden stationaries: layer s (1..4) consumes h_{s-1} laid
    # out per ROT[s-1]; stationary row r multiplies h row r
    whp = np.zeros((128, N_HIDDEN, HID), np.float32)
    for s in range(1, N_STAGES):
        rowmap = _hid_of_row(s - 1)
        for r in range(128):
            hid = rowmap[r]
            if hid is not None:
                whp[r, s - 1, :] = W_h[s - 1, hid, :]

    # placed biases [128, 5]
    b_in = np.asarray(b_in, np.float32)
    b_h = np.asarray(b_h, np.float32)  # [4, 90]
    bia = np.zeros((128, N_STAGES), np.float32)
    for s in range(N_STAGES):
        rowmap = _hid_of_row(s)
        bs = b_in if s == 0 else b_h[s - 1]
        for r in range(128):
            if rowmap[r] is not None:
                bia[r, s] = bs[rowmap[r]]

    common = {
        "wz": np.ascontiguousarray(
            np.concatenate([Wd, Wc], axis=0).astype(bf)
        ),
        "wh": np.ascontiguousarray(whp.astype(bf)),
        "wo": np.ascontiguousarray(np.asarray(W_out, np.float32).astype(bf)),
        "bia": np.ascontiguousarray(bia),
        "boc": np.full((97, 1), np.asarray(b_out, np.float32).ravel()[0],
                       dtype=np.float32),
    }
    return common


def _pack_x(x_core, ctl_rows):
    b = x_core.shape[0]
    p = x_core.reshape(b, 32, 2, 32, 2).transpose(2, 4, 0, 1, 3)
    out = np.empty((7, b * N_NODES), dtype=ml_dtypes.bfloat16)
    out[0:4] = p.reshape(4, b * N_NODES).astype(ml_dtypes.bfloat16)
    out[4:7] = np.tile(ctl_rows, (1, b))
    return np.ascontiguousarray(out)


def _run(inputs, trace=False, tmpdir=None):
    x = np.asarray(inputs["x"], dtype=np.float32)
    common = _prep_weights(
        inputs["controls"], inputs["W_in"], inputs["b_in"], inputs["W_h"],
        inputs["b_h"], inputs["W_out"], inputs["b_out"],
        inputs["data_idx"], inputs["ctrl_idx"],
    )
    ctl_rows = (
        np.asarray(inputs["controls"], np.float32).T.astype(ml_dtypes.bfloat16)
    )

    nc = _build_program(B_CORE, CHUNK_B)

    core_ids = list(range(N_CORES))
    in_maps = []
    for i in core_ids:
        shard = _pack_x(x[i * B_CORE : (i + 1) * B_CORE, 0], ctl_rows)
        in_maps.append({"xs": shard, **common})

    res = run_bass_kernel_spmd(nc, in_maps, core_ids, trace=trace, tmpdir=tmpdir)
    out = np.concatenate([res.results[i]["out"] for i in core_ids], axis=0)
    return out.reshape(B, 32, 32), res.exec_time_ns


def kernel(**inputs):
    return _run(inputs, trace=False)[0]


# revision 17
# speedup vs baseline: 1.2661x; 1.0949x over previous
"""Trainium2 Bass kernel for the DNPU local-receptive-field surrogate model.

Model (see reference): x [B,1,64,64] -> 2x2/stride-2 unfold -> per-node
7-electrode assembly -> shared MLP 7->90->(90x4)->1 -> out [B,32,32].

Measured hardware facts driving the design (this axon-tunneled TRN2):
  - A bf16 matmul streams 1 moving column per 1.2GHz cycle (426.7ns per
    512-column tile), regardless of dtype, M, K, and runtime (no 2.4GHz
    p-state ever engages; verified with 2000 dependency-free matmuls).
  - Matmuls whose outputs sit in DIFFERENT 32-partition PSUM column
    blocks (matmul tile_position=(0, 32j)) execute CONCURRENTLY; tiles
    sharing a column block serialize. So PE throughput = columns per
    block-saturated schedule, NOT per instruction.
  - ACT/DVE (the only PSUM readers) are free-dim-serial: a [128,512]
    drain costs the same as [1,512] (~620ns), so drains want ALL used
    partitions per instruction.
  - fp8 DoubleRow is ISA-rejected for this geometry and ~12% rel err
    anyway (budget 2e-2); bf16 keeps rel err at 6.4e-3.

Strategy:
  - Data-parallel over batch: 64 batches/core x 8 cores; weights and
    per-node controls replicated; host pre-packs x into pixel-major
    [7, tokens] (4 patch pixels + 3 tiled control rows) so unfold +
    electrode scatter + layer-0 is one K=7 matmul per 512-token tile.
  - SOFTWARE-PIPELINED BLOCK-ROTATED SWEEPS: each relu layer s places
    its 90 output rows on 3 of the 4 PSUM column blocks, rotated per
    layer (ROT[s]); the next layer's stationary is host-permuted to
    [128, 90] with zero rows where the producer layout has garbage, so
    K=128 matmuls consume the placed layout directly and a single
    [128, 512] drain (placed bias + relu; junk rows drain harmlessly,
    zero stationary rows kill them downstream) retires each tile.
    A per-chunk round-robin emitter interleaves all 5 layer sweeps
    (+ the M=1 out matmuls, 4 per block-packed PSUM slot) with a lag,
    so on average all 4 column blocks stay loaded: per-block demand is
    15 layer-parts + out spread over 4 blocks -> ~342ns/tile-layer vs
    427 serialized, on top of the out layer costing ~1/4 of a sweep.
  - Out groups: 4 M=1 matmuls -> partitions {0,32,64,96} of one PSUM
    slot -> one [97,512] drain -> one 4-partition-line DMA (the old
    [1, chunk_tok] single-line output DMA was a ~25us serial transfer
    that dominated the tail).
  - Chunk-0 x arrives in staged pieces so layer 0 starts once ~28KB
    land; later chunks prefetch after layer 0 stops reading xt.
"""

import ml_dtypes
import numpy as np

import concourse.bass as bass
import concourse.mybir as mybir
import concourse.tile as _tile
from concourse.bass_utils import run_bass_kernel_spmd

# ---------------------------------------------------------------------------
# Workaround: this neuronxcc walrus build rejects instructions carrying more
# than one sem wait ("Too many sync wait commands"). Spill excess waits onto
# NOPs inserted just before the instruction on the same engine.
_MAX_SYNC_WAITS = 1
_nop_counter = [0]


def _split_excess_sync_waits(nc, maxw=_MAX_SYNC_WAITS):
    for f in nc.m.functions:
        for bb in f.blocks:
            insts = list(bb.instructions)
            if not any(
                ins.sync_info is not None and len(ins.sync_info.on_wait or []) > maxw
                for ins in insts
            ):
                continue
            new = []
            for ins in insts:
                si = ins.sync_info
                waits = list(si.on_wait or []) if si is not None else []
                if len(waits) > maxw:
                    excess, keep = waits[: len(waits) - maxw], waits[-maxw:]
                    for i in range(0, len(excess), maxw):
                        _nop_counter[0] += 1
                        nop = mybir.InstNoOp(name=f"waitsplit_{_nop_counter[0]}")
                        nop.engine = ins.engine
                        nop.sync_info = mybir.SyncInfo(
                            on_wait=excess[i : i + maxw], on_update=[]
                        )
                        new.append(nop)
                    si.on_wait = keep
                new.append(ins)
            bb.instructions = new

# ---------------------------------------------------------------------------
B = 512
H = W = 64
K = 2
N_NODES = (H // K) * (W // K)  # 1024
HID = 90
N_HIDDEN = 4
N_STAGES = 1 + N_HIDDEN  # relu layers (l0..l4)
N_CORES = 8
B_CORE = B // N_CORES  # 64 batches per core

CHUNK_B = 16
N_TILE = 512
OGRP = 4  # out tiles per PSUM slot

# per-relu-layer column blocks for parts P0 (hid 0:32), P1 (32:64),
# P2 (64:90); rotated so the 15 parts spread over 4 blocks
ROT = [[0, 1, 2], [3, 0, 1], [2, 3, 0], [1, 2, 3], [0, 1, 2]]
PART_W = [32, 32, HID - 64]  # part widths
PART_H0 = [0, 32, 64]  # hid range starts
LAG_T = 5  # consumer stage trails producer by this many tiles

F32 = mybir.dt.float32
BF16 = mybir.dt.bfloat16

_COST = {
    "act_out": 612.0,
    "dve_out": 700.0,  # measured 658 + DVE sem/NOP overhead share
}


def _hid_of_row(s):
    """For relu layer s's output layout: psum row r -> hid index or None."""
    m = [None] * 128
    for j, blk in enumerate(ROT[s]):
        for k in range(PART_W[j]):
            m[32 * blk + k] = PART_H0[j] + k
    return m


def _build_program(b_core: int, chunk_b: int):
    nc = bass.Bass()

    n_chunks = b_core // chunk_b
    chunk_tok = chunk_b * N_NODES
    tiles = chunk_tok // N_TILE  # 32
    ogroups = tiles // OGRP  # 8

    xs_d = nc.dram_tensor("xs", [7, b_core * N_NODES], BF16, kind="ExternalInput")
    wz_d = nc.dram_tensor("wz", [7, HID], BF16, kind="ExternalInput")
    # permuted hidden stationaries [128, 4, 90]
    wh_d = nc.dram_tensor("wh", [128, N_HIDDEN, HID], BF16, kind="ExternalInput")
    wo_d = nc.dram_tensor("wo", [HID, 1], BF16, kind="ExternalInput")
    # placed biases [128, 5] (col s = relu layer s, rows per ROT[s])
    bia_d = nc.dram_tensor("bia", [128, N_STAGES], F32, kind="ExternalInput")
    boc_d = nc.dram_tensor("boc", [97, 1], F32, kind="ExternalInput")
    z_d = nc.dram_tensor("z", [1, 640], BF16, kind="ExternalInput")
    out_d = nc.dram_tensor("out", [b_core, N_NODES], F32, kind="ExternalOutput")

    Relu = mybir.ActivationFunctionType.Relu
    Identity = mybir.ActivationFunctionType.Identity
    ALU_ADD = mybir.AluOpType.add
    ALU_MAX = mybir.AluOpType.max

    eng_t = [0.0, 0.0]

    with _tile.TileContext(nc) as tc:
        with (
            tc.tile_pool(name="const", bufs=1) as const,
            tc.tile_pool(name="xin", bufs=1) as xin,
            tc.tile_pool(name="outp", bufs=2 * ogroups) as outp,
            tc.tile_pool(name="hbuf", bufs=60) as hbuf,
            tc.tile_pool(name="ps", bufs=8, space="PSUM") as ps,
        ):
            zt = const.tile([1, 640], BF16)
            nc.sync.dma_start(zt[:], z_d[:])
            wz = const.tile([7, HID], BF16)
            nc.sync.dma_start(wz[:], wz_d[:])
            # double-buffered chunk input; chunk 0 staged in pieces so
            # layer 0 starts as soon as ~28KB land
            xts = []
            for cb in range(2):
                xts.append(xin.tile([7, chunk_tok], BF16, name=f"xt{cb}"))
            bounds = [0, 4, 8, 14, 22, 32]
            for a, b in zip(bounds, bounds[1:]):
                nc.sync.dma_start(
                    xts[0][:, a * N_TILE : b * N_TILE],
                    xs_d[:, a * N_TILE : b * N_TILE],
                )
            nc.sync.dma_start(xts[1][:], xs_d[:, chunk_tok : 2 * chunk_tok])
            bia = const.tile([128, N_STAGES], F32)
            nc.sync.dma_start(bia[:], bia_d[:])
            wh = const.tile([128, N_HIDDEN, HID], BF16)
            nc.sync.dma_start(wh[:], wh_d[:])
            wo = const.tile([HID, 1], BF16)
            nc.sync.dma_start(wo[:], wo_d[:])
            boc = const.tile([97, 1], F32)
            nc.sync.dma_start(boc[:], boc_d[:])

            # scrub all 8 PSUM ring slots to 0 before any [128,512] drain
            # can observe boot-time garbage (0 x NaN = NaN would otherwise
            # poison the zero-stationary-row trick); overlaps the x DMA.
            for _ in range(8):
                pz = ps.tile([128, N_TILE], F32, tag="ps", name="pz")
                nc.tensor.matmul(
                    pz[:], zt[0:1, 0:128], zt[0:1, 128 : 128 + N_TILE]
                )

            def drain(dst, src, bias_ap, relu, eng=None):
                ca, cd = _COST["act_out"], _COST["dve_out"]
                if eng is None:
                    use_act = eng_t[0] + ca <= eng_t[1] + cd
                else:
                    use_act = eng == 0
                if use_act:
                    eng_t[0] += ca
                    nc.scalar.activation(
                        dst, src, Relu if relu else Identity, bias=bias_ap
                    )
                elif relu:
                    eng_t[1] += cd
                    nc.vector.tensor_scalar(
                        out=dst, in0=src, scalar1=bias_ap, scalar2=0.0,
                        op0=ALU_ADD, op1=ALU_MAX,
                    )
                else:
                    eng_t[1] += cd
                    nc.vector.tensor_scalar(
                        out=dst, in0=src, scalar1=bias_ap, scalar2=None,
                        op0=ALU_ADD,
                    )

            # ---- single global software-pipelined emission over all
            # chunks: stage s's tile pointer runs over 0..n_tiles_total,
            # trailing its producer by LAG_T tiles; l0 switches xt
            # buffers at chunk boundaries and triggers the next
            # prefetch as soon as it finishes reading a chunk.
            n_tiles_total = n_chunks * tiles
            n_ogroups_total = n_tiles_total // OGRP
            hcur = [[None] * n_tiles_total for _ in range(N_STAGES)]

            def emit_stage_tile(s, t):
                """3 part-matmuls + 1 drain for relu layer s, tile t."""
                pt = ps.tile([128, N_TILE], F32, tag="ps", name="pt_s")
                for j, blk in enumerate(ROT[s]):
                    w = PART_W[j]
                    if s == 0:
                        lhs = wz[:, PART_H0[j] : PART_H0[j] + w]
                        tl = t % tiles
                        rhs = xts[(t // tiles) % 2][
                            :, tl * N_TILE : (tl + 1) * N_TILE
                        ]
                    else:
                        lhs = wh[:, s - 1, PART_H0[j] : PART_H0[j] + w]
                        rhs = hcur[s - 1][t][:]
                    nc.tensor.matmul(
                        pt[32 * blk : 32 * blk + w, :],
                        lhs,
                        rhs,
                        tile_position=(0, 32 * blk),
                    )
                h = hbuf.tile([128, N_TILE], BF16, tag="h", name="h_s")
                drain(h[:], pt[:], bia[:, s : s + 1], relu=True)
                hcur[s][t] = h

            def emit_out_group(g):
                """4 M=1 out matmuls -> one psum slot -> drain + DMA."""
                pt = ps.tile([128, N_TILE], F32, tag="ps", name="pt_o")
                for j in range(OGRP):
                    t = g * OGRP + j
                    nc.tensor.matmul(
                        pt[32 * j : 32 * j + 1, :],
                        wo[:],
                        hcur[N_STAGES - 1][t][0:HID, :],
                        tile_position=(0, 32 * j),
                    )
                og = outp.tile([97, N_TILE], F32, tag="o", name="og")
                drain(og[:], pt[0:97, :], boc[:], relu=False)
                dst = (
                    out_d[2 * g : 2 * g + 2]
                    .rearrange("b (r c) -> (b r) c", r=2, c=N_TILE)
                )
                nc.sync.dma_start(dst, og[0:97:32, :])

            p = [0] * N_STAGES
            po = 0
            next_prefetch = 2  # chunks 0,1 already issued
            while p[-1] < n_tiles_total or po < n_ogroups_total:
                progress = False
                for s in range(N_STAGES):
                    limit = n_tiles_total if s == 0 else p[s - 1] - LAG_T
                    if p[s] < n_tiles_total and p[s] < limit:
                        emit_stage_tile(s, p[s])
                        p[s] += 1
                        progress = True
                        if (
                            s == 0
                            and next_prefetch < n_chunks
                            and p[0] == (next_prefetch - 1) * tiles
                        ):
                            # l0 done reading chunk next_prefetch-2's
                            # buffer; refill it with chunk next_prefetch
                            nt0 = next_prefetch * chunk_tok
                            nc.sync.dma_start(
                                xts[next_prefetch % 2][:],
                                xs_d[:, nt0 : nt0 + chunk_tok],
                            )
                            next_prefetch += 1
                if po < n_ogroups_total and (po + 1) * OGRP <= p[-1] - LAG_T:
                    emit_out_group(po)
                    po += 1
                    progress = True
                if not progress:
                    for s in range(N_STAGES):
                        if p[s] < n_tiles_total and (
                            s == 0 or p[s] < p[s - 1]
                        ):
                            emit_stage_tile(s, p[s])
                            p[s] += 1
                            break
                    else:
                        if po < n_ogroups_total:
                            emit_out_group(po)
                            po += 1

    _split_excess_sync_waits(nc)
    return nc


def _prep_weights(controls, W_in, b_in, W_h, b_h, W_out, b_out, data_idx, ctrl_idx):
    di = np.asarray(data_idx)[0].tolist()
    ci = np.asarray(ctrl_idx)[0].tolist()
    W_in = np.asarray(W_in, dtype=np.float32)
    Wd = W_in[di, :].copy()
    cset = set(ci)
    for j in range(4):
        if di[j] in cset or di[j] in di[j + 1 :]:
            Wd[j] = 0.0
    Wc = W_in[ci, :].copy()
    for k in range(3):
        if ci[k] in ci[k + 1 :]:
            Wc[k] = 0.0

    bf = ml_dtypes.bfloat16
    W_h = np.asarray(W_h, np.float32)  # [4, 90, 90]

    # permuted hidden stationaries: layer s (1..4) consumes h_{s-1} laid
    # out per ROT[s-1]; stationary row r multiplies h row r
    whp = np.zeros((128, N_HIDDEN, HID), np.float32)
    for s in range(1, N_STAGES):
        rowmap = _hid_of_row(s - 1)
        for r in range(128):
            hid = rowmap[r]
            if hid is not None:
                whp[r, s - 1, :] = W_h[s - 1, hid, :]

    # placed biases [128, 5]
    b_in = np.asarray(b_in, np.float32)
    b_h = np.asarray(b_h, np.float32)  # [4, 90]
    bia = np.zeros((128, N_STAGES), np.float32)
    for s in range(N_STAGES):
        rowmap = _hid_of_row(s)
        bs = b_in if s == 0 else b_h[s - 1]
        for r in range(128):
            if rowmap[r] is not None:
                bia[r, s] = bs[rowmap[r]]

    common = {
        "wz": np.ascontiguousarray(
            np.concatenate([Wd, Wc], axis=0).astype(bf)
        ),
        "wh": np.ascontiguousarray(whp.astype(bf)),
        "wo": np.ascontiguousarray(np.asarray(W_out, np.float32).astype(bf)),
        "bia": np.ascontiguousarray(bia),
        "boc": np.full((97, 1), np.asarray(b_out, np.float32).ravel()[0],
                       dtype=np.float32),
        "z": np.zeros((1, 640), dtype=ml_dtypes.bfloat16),
    }
    return common


def _pack_x(x_core, ctl_rows):
    b = x_core.shape[0]
    p = x_core.reshape(b, 32, 2, 32, 2).transpose(2, 4, 0, 1, 3)
    out = np.empty((7, b * N_NODES), dtype=ml_dtypes.bfloat16)
    out[0:4] = p.reshape(4, b * N_NODES).astype(ml_dtypes.bfloat16)
    out[4:7] = np.tile(ctl_rows, (1, b))
    return np.ascontiguousarray(out)


def _run(inputs, trace=False, tmpdir=None):
    x = np.asarray(inputs["x"], dtype=np.float32)
    common = _prep_weights(
        inputs["controls"], inputs["W_in"], inputs["b_in"], inputs["W_h"],
        inputs["b_h"], inputs["W_out"], inputs["b_out"],
        inputs["data_idx"], inputs["ctrl_idx"],
    )
    ctl_rows = (
        np.asarray(inputs["controls"], np.float32).T.astype(ml_dtypes.bfloat16)
    )

    nc = _build_program(B_CORE, CHUNK_B)

    core_ids = list(range(N_CORES))
    in_maps = []
    for i in core_ids:
        shard = _pack_x(x[i * B_CORE : (i + 1) * B_CORE, 0], ctl_rows)
        in_maps.append({"xs": shard, **common})

    res = run_bass_kernel_spmd(nc, in_maps, core_ids, trace=trace, tmpdir=tmpdir)
    out = np.concatenate([res.results[i]["out"] for i in core_ids], axis=0)
    return out.reshape(B, 32, 32), res.exec_time_ns


def kernel(**inputs):
    return _run(inputs, trace=False)[0]
